# revision 3
# baseline (speedup 1.0000x reference)
"""MegablockMoE kernel for 8 Trainium2 NeuronCores.

Strategy (per sharding hint): expert-parallel. The router + token
dispatch/combine permutations (pure index bookkeeping) run on host as the
shard/unshard step; each of the 8 cores owns one expert and runs the two big
GEMMs (gelu(xg @ w1[e]) @ w2[e], 34.4 GFLOP/core) in bf16 with fp32 PSUM
accumulation, weights resident in SBUF, hT intermediate never leaving chip.

Device kernel (identical NEFF on all 8 cores, SPMD over experts):
    in : xgT [D, C] bf16   -- gathered tokens for this expert, transposed
         w1  [D, DFF] bf16, w2 [DFF, D] bf16
    mid: hT  [DFF, c_tile] bf16 = gelu(w1.T @ xgT)   (exact erf gelu, SBUF)
    out: yT  [D, C] f32    = w2.T @ hT
"""

import numpy as np
import ml_dtypes

import concourse.mybir as mybir
import concourse.tile as tile
from concourse import bacc
from concourse.bass_utils import run_bass_kernel_spmd

B, S, D = 4, 2048, 1024
E, K, DFF = 8, 2, 4096
T = B * S
C = K * T // E  # 2048 expert capacity
BF16 = ml_dtypes.bfloat16
N_CORES = 8

KO1, KO2 = D // 128, DFF // 128
W1_CH = 8            # f-chunks of w1 (separate tiles -> fine-grained DMA deps)
W1_F = DFF // W1_CH  # 512
W2_CH = 8            # o-chunks of w2
W2_O = KO2 // W2_CH  # 4

_NC = None


def _build_nc(c_tile=256, psum_bufs=3, ht_bufs=2, y_bufs=2, xg_bufs=2,
              n_iters=1, debug=True):
    nc = bacc.Bacc(None, target_bir_lowering=False, debug=debug)
    xgT = nc.dram_tensor("xgT", [D, C], mybir.dt.bfloat16, kind="ExternalInput")
    w1 = nc.dram_tensor("w1", [D, DFF], mybir.dt.bfloat16, kind="ExternalInput")
    w2 = nc.dram_tensor("w2", [DFF, D], mybir.dt.bfloat16, kind="ExternalInput")
    yT = nc.dram_tensor("yT", [D, C], mybir.dt.float32, kind="ExternalOutput")

    xgT_v = xgT.rearrange("(o p) c -> p o c", p=128)
    w1_v = w1.rearrange("(o p) f -> p o f", p=128)
    w2_v = w2.rearrange("(o p) d -> p o d", p=128)
    yT_v = yT.rearrange("(o p) c -> p o c", p=128)
    n_ct = C // c_tile

    with tile.TileContext(nc) as tc:
        with (
            tc.tile_pool(name="wpool", bufs=1) as wpool,
            tc.tile_pool(name="xpool", bufs=xg_bufs) as xpool,
            tc.tile_pool(name="hpool", bufs=ht_bufs) as hpool,
            tc.tile_pool(name="ypool", bufs=y_bufs) as ypool,
            tc.tile_pool(name="ps1", bufs=psum_bufs, space="PSUM") as ps1,
            tc.tile_pool(name="ps2", bufs=psum_bufs, space="PSUM") as ps2,
        ):
            # first xg tile before weights: small and needed immediately
            xg_tiles = {}
            if n_iters == 1:
                xg_tiles[0] = xpool.tile([128, KO1, c_tile], mybir.dt.bfloat16,
                                         tag="xg", name="xg0")
                nc.sync.dma_start(xg_tiles[0][:], xgT_v[:, :, 0:c_tile])

            w1_tiles = []
            for ch in range(W1_CH):
                wt = wpool.tile([128, KO1, W1_F], mybir.dt.bfloat16,
                                tag=f"w1_{ch}", name=f"w1t{ch}")
                nc.sync.dma_start(wt[:], w1_v[:, :, ch * W1_F : (ch + 1) * W1_F])
                w1_tiles.append(wt)
            w2_tiles = []
            for ch in range(W2_CH):
                wt = wpool.tile([128, W2_O, D], mybir.dt.bfloat16,
                                tag=f"w2_{ch}", name=f"w2t{ch}")
                nc.sync.dma_start(wt[:], w2_v[:, ch * W2_O : (ch + 1) * W2_O, :])
                w2_tiles.append(wt)

            def w1_ap(o, f):
                ch, r = divmod(f, W1_F // 128)
                return w1_tiles[ch][:, o, r * 128 : (r + 1) * 128]

            def w2_ap(f, g):
                ch, r = divmod(f, W2_O)
                return w2_tiles[ch][:, r, g * 128 : (g + 1) * 128]

            def body(_=None):
                for t in range(n_ct):
                    cs = slice(t * c_tile, (t + 1) * c_tile)
                    if t not in xg_tiles:
                        xg_tiles[t] = xpool.tile(
                            [128, KO1, c_tile], mybir.dt.bfloat16, tag="xg",
                            name=f"xg{t}",
                        )
                        nc.sync.dma_start(xg_tiles[t][:], xgT_v[:, :, cs])
                    xg_sb = xg_tiles[t]

                    hT_sb = hpool.tile([128, KO2, c_tile], mybir.dt.bfloat16,
                                       tag="hT")
                    for f in range(KO2):
                        psum = ps1.tile([128, c_tile], mybir.dt.float32,
                                        tag="p1")
                        for o in range(KO1):
                            nc.tensor.matmul(
                                psum[:], w1_ap(o, f), xg_sb[:, o, :],
                                start=(o == 0), stop=(o == KO1 - 1),
                            )
                        nc.scalar.activation(
                            hT_sb[:, f, :], psum[:],
                            mybir.ActivationFunctionType.Gelu,
                        )

                    y_sb = ypool.tile([128, KO1, c_tile], mybir.dt.float32,
                                      tag="y")
                    for g in range(KO1):
                        psum = ps2.tile([128, c_tile], mybir.dt.float32,
                                        tag="p2")
                        for f in range(KO2):
                            nc.tensor.matmul(
                                psum[:], w2_ap(f, g), hT_sb[:, f, :],
                                start=(f == 0), stop=(f == KO2 - 1),
                            )
                        nc.vector.tensor_copy(y_sb[:, g, :], psum[:])
                    nc.sync.dma_start(yT_v[:, :, cs], y_sb[:])

            if n_iters == 1:
                body()
            else:
                with tc.For_i(0, n_iters, 1):
                    body()
    nc.compile()
    return nc


def _get_nc():
    global _NC
    if _NC is None:
        _NC = _build_nc()
    return _NC


def _route(x, wr):
    """Replicates the reference router exactly (fp32 numpy)."""
    xt = np.transpose(x, (1, 0, 2)).reshape(T, D)  # [T, D] fp32
    logits = xt.astype(np.float32) @ wr.astype(np.float32)  # [T, E]
    m = logits.max(axis=-1, keepdims=True)
    p = np.exp(logits - m, dtype=np.float32)
    p /= p.sum(axis=-1, keepdims=True)
    top1 = np.argmax(p, axis=-1)
    p_masked = p.copy()
    p_masked[np.arange(T), top1] = -np.inf
    top2 = np.argmax(p_masked, axis=-1)
    eidx = np.stack([top1, top2], axis=1)  # [T, K]
    ew = np.take_along_axis(p, eidx, axis=1).astype(np.float32)  # [T, K]

    flat_e = eidx.reshape(-1)
    order = np.argsort(flat_e, kind="stable")
    sorted_e = flat_e[order]
    hist = np.bincount(flat_e, minlength=E)
    starts = np.cumsum(hist) - hist
    pos = np.arange(T * K) - starts[sorted_e]
    keep = pos < C
    slot = np.where(keep, sorted_e * C + pos, E * C)
    token = order // K
    return xt, ew, order, keep, slot, token


def _make_in_maps(x, wr, w1, w2):
    xt, ew, order, keep, slot, token = _route(x, wr)
    slot_token = np.zeros(E * C, np.int64)
    slot_token[slot[keep]] = token[keep]
    xT_bf = np.ascontiguousarray(xt.T.astype(BF16))  # [D, T]
    in_maps = []
    for e in range(E):
        idx = slot_token[e * C : (e + 1) * C]
        in_maps.append(
            {
                "xgT": np.ascontiguousarray(xT_bf[:, idx]),
                "w1": np.ascontiguousarray(w1[e].astype(BF16)),
                "w2": np.ascontiguousarray(w2[e].astype(BF16)),
            }
        )
    return in_maps, (ew, order, keep, slot)


def kernel(x, wr, w1, w2):
    nc = _get_nc()
    in_maps, (ew, order, keep, slot) = _make_in_maps(x, wr, w1, w2)

    res = run_bass_kernel_spmd(nc, in_maps, core_ids=list(range(N_CORES)))

    # --- combine: weighted scatter back to tokens ---
    Y = np.empty((E * C, D), np.float32)
    for e in range(E):
        Y[e * C : (e + 1) * C] = res.results[e]["yT"].T

    inv = np.empty(T * K, np.int64)
    inv[order] = np.arange(T * K)
    slot_tk = slot[inv].reshape(T, K)
    keep_tk = keep[inv].reshape(T, K)

    out_flat = np.zeros((T, D), np.float32)
    for k in range(K):
        sl = np.clip(slot_tk[:, k], 0, E * C - 1)
        contrib = Y[sl] * ew[:, k : k + 1]
        contrib[~keep_tk[:, k]] = 0.0
        out_flat += contrib
    return np.ascontiguousarray(
        out_flat.reshape(S, B, D).transpose(1, 0, 2)
    ).astype(np.float32)


# ---------------------------------------------------------------------------
# Benchmark helper (used by test.py; not part of the grading contract).
# ---------------------------------------------------------------------------


def make_bench(in_maps):
    import jax
    from jax.experimental.shard_map import shard_map
    from jax.sharding import Mesh, PartitionSpec, NamedSharding
    from concourse.bass2jax import (
        _bass_exec_p,
        install_neuronx_cc_hook,
        partition_id_tensor,
    )

    nc = _NC if _NC is not None else _get_nc()
    install_neuronx_cc_hook()
    partition_name = nc.partition_id_tensor.name if nc.partition_id_tensor else None

    in_names, out_names, out_avals, zero_outs = [], [], [], []
    for alloc in nc.m.functions[0].allocations:
        if not isinstance(alloc, mybir.MemoryLocationSet):
            continue
        name = alloc.memorylocations[0].name
        if alloc.kind == "ExternalInput":
            if name != partition_name:
                in_names.append(name)
        elif alloc.kind == "ExternalOutput":
            shape = tuple(alloc.tensor_shape)
            dtype = mybir.dt.np(alloc.dtype)
            out_avals.append(jax.core.ShapedArray(shape, dtype))
            zero_outs.append(np.zeros(shape, dtype))
            out_names.append(name)
    n_params = len(in_names)
    all_in_names = list(in_names) + list(out_names)
    if partition_name is not None:
        all_in_names.append(partition_name)
    if nc.dbg_addr is not None:
        dbg_zero = np.zeros((1, 2), np.uint32)
        in_maps = [{**m, nc.dbg_addr.name: dbg_zero} for m in in_maps]

    def _body(*args):
        operands = list(args)
        if partition_name is not None:
            operands.append(partition_id_tensor())
        outs = _bass_exec_p.bind(
            *operands,
            out_avals=tuple(out_avals),
            in_names=tuple(all_in_names),
            out_names=tuple(out_names),
            lowering_input_output_aliases=(),
            sim_require_finite=True,
            sim_require_nnan=True,
            nc=nc,
        )
        return tuple(outs)

    devices = jax.devices()[:N_CORES]
    mesh = Mesh(np.asarray(devices), ("core",))
    n_outs = len(out_names)
    in_specs = (PartitionSpec("core"),) * (n_params + n_outs)
    out_specs = (PartitionSpec("core"),) * n_outs
    fn = jax.jit(
        shard_map(_body, mesh=mesh, in_specs=in_specs, out_specs=out_specs,
                  check_rep=False),
        keep_unused=True,
    )
    concat_in = [
        np.concatenate([np.asarray(in_maps[c][name]) for c in range(N_CORES)],
                       axis=0)
        for name in in_names
    ]
    concat_zeros = [
        np.zeros((N_CORES * z.shape[0], *z.shape[1:]), z.dtype)
        for z in zero_outs
    ]
    shard = NamedSharding(mesh, PartitionSpec("core"))
    args = [jax.device_put(a, shard) for a in concat_in + concat_zeros]
    return fn, args, out_names


def benchmark(in_maps, iters=20, warmup=3):
    import time
    import jax

    fn, args, out_names = make_bench(in_maps)
    for _ in range(warmup):
        out = fn(*args)
        jax.block_until_ready(out)
    times = []
    for _ in range(iters):
        t0 = time.perf_counter()
        out = fn(*args)
        jax.block_until_ready(out)
        times.append(time.perf_counter() - t0)
    return min(times), sorted(times)[len(times) // 2], out


# revision 5
# speedup vs baseline: 184.1186x; 184.1186x over previous
"""MegablockMoE kernel for 8 Trainium2 NeuronCores.

Strategy (per sharding hint): expert-parallel. The router + token
dispatch/combine permutations (pure index bookkeeping) run on host as the
shard/unshard step; each of the 8 cores owns one expert and runs the two big
GEMMs (gelu(xg @ w1[e]) @ w2[e], 34.4 GFLOP/core) in bf16 with fp32 PSUM
accumulation, weights resident in SBUF, hT intermediate never leaving chip.

Device kernel (identical NEFF on all 8 cores, SPMD over experts):
    in : xgT [D, C] bf16   -- gathered tokens for this expert, transposed
         w1  [D, DFF] bf16, w2 [DFF, D] bf16
    mid: hT  [DFF, c_tile] bf16 = gelu(w1.T @ xgT)   (exact erf gelu, SBUF)
    out: yT  [D, C] f32    = w2.T @ hT
"""

import numpy as np
import ml_dtypes

import concourse.mybir as mybir
import concourse.tile as tile
from concourse import bacc
from concourse.bass_utils import run_bass_kernel_spmd

B, S, D = 4, 2048, 1024
E, K, DFF = 8, 2, 4096
T = B * S
C = K * T // E  # 2048 expert capacity
BF16 = ml_dtypes.bfloat16
N_CORES = 8

KO1, KO2 = D // 128, DFF // 128
W1_CH = 8            # f-chunks of w1 (separate tiles -> fine-grained DMA deps)
W1_F = DFF // W1_CH  # 512
W2_CH = 8            # o-chunks of w2
W2_O = KO2 // W2_CH  # 4

_NC = None


def _build_nc(c_tile=256, psum_bufs=4, ht_bufs=2, y_bufs=2, xg_bufs=2,
              n_iters=1, debug=True):
    nc = bacc.Bacc(None, target_bir_lowering=False, debug=debug)
    xgT = nc.dram_tensor("xgT", [D, C], mybir.dt.bfloat16, kind="ExternalInput")
    w1 = nc.dram_tensor("w1", [D, DFF], mybir.dt.bfloat16, kind="ExternalInput")
    w2 = nc.dram_tensor("w2", [DFF, D], mybir.dt.bfloat16, kind="ExternalInput")
    yT = nc.dram_tensor("yT", [D, C], mybir.dt.float32, kind="ExternalOutput")

    xgT_v = xgT.rearrange("(o p) c -> p o c", p=128)
    w1_v = w1.rearrange("(o p) f -> p o f", p=128)
    w2_v = w2.rearrange("(o p) d -> p o d", p=128)
    yT_v = yT.rearrange("(o p) c -> p o c", p=128)
    n_ct = C // c_tile

    with tile.TileContext(nc) as tc:
        with (
            tc.tile_pool(name="wpool", bufs=1) as wpool,
            tc.tile_pool(name="xpool", bufs=xg_bufs) as xpool,
            tc.tile_pool(name="hpool", bufs=ht_bufs) as hpool,
            tc.tile_pool(name="ypool", bufs=y_bufs) as ypool,
            tc.tile_pool(name="ps1", bufs=psum_bufs, space="PSUM") as ps1,
            tc.tile_pool(name="ps2", bufs=psum_bufs, space="PSUM") as ps2,
        ):
            # first xg tile before weights: small and needed immediately
            xg_tiles = {}
            if n_iters == 1:
                xg_tiles[0] = xpool.tile([128, KO1, c_tile], mybir.dt.bfloat16,
                                         tag="xg", name="xg0")
                nc.sync.dma_start(xg_tiles[0][:], xgT_v[:, :, 0:c_tile])

            w1_tiles = []
            for ch in range(W1_CH):
                wt = wpool.tile([128, KO1, W1_F], mybir.dt.bfloat16,
                                tag=f"w1_{ch}", name=f"w1t{ch}")
                nc.sync.dma_start(wt[:], w1_v[:, :, ch * W1_F : (ch + 1) * W1_F])
                w1_tiles.append(wt)
            w2_tiles = []
            for ch in range(W2_CH):
                wt = wpool.tile([128, W2_O, D], mybir.dt.bfloat16,
                                tag=f"w2_{ch}", name=f"w2t{ch}")
                nc.sync.dma_start(wt[:], w2_v[:, ch * W2_O : (ch + 1) * W2_O, :])
                w2_tiles.append(wt)

            def w1_ap(o, f):
                ch, r = divmod(f, W1_F // 128)
                return w1_tiles[ch][:, o, r * 128 : (r + 1) * 128]

            def w2_ap(f, g):
                ch, r = divmod(f, W2_O)
                return w2_tiles[ch][:, r, g * 128 : (g + 1) * 128]

            def body(_=None):
                for t in range(n_ct):
                    cs = slice(t * c_tile, (t + 1) * c_tile)
                    if t not in xg_tiles:
                        xg_tiles[t] = xpool.tile(
                            [128, KO1, c_tile], mybir.dt.bfloat16, tag="xg",
                            name=f"xg{t}",
                        )
                        nc.sync.dma_start(xg_tiles[t][:], xgT_v[:, :, cs])
                    xg_sb = xg_tiles[t]

                    hT_sb = hpool.tile([128, KO2, c_tile], mybir.dt.bfloat16,
                                       tag="hT")
                    for f in range(KO2):
                        psum = ps1.tile([128, c_tile], mybir.dt.float32,
                                        tag="p1")
                        for o in range(KO1):
                            nc.tensor.matmul(
                                psum[:], w1_ap(o, f), xg_sb[:, o, :],
                                start=(o == 0), stop=(o == KO1 - 1),
                            )
                        nc.scalar.activation(
                            hT_sb[:, f, :], psum[:],
                            mybir.ActivationFunctionType.Gelu,
                        )

                    y_sb = ypool.tile([128, KO1, c_tile], mybir.dt.float32,
                                      tag="y")
                    for g in range(KO1):
                        psum = ps2.tile([128, c_tile], mybir.dt.float32,
                                        tag="p2")
                        for f in range(KO2):
                            nc.tensor.matmul(
                                psum[:], w2_ap(f, g), hT_sb[:, f, :],
                                start=(f == 0), stop=(f == KO2 - 1),
                            )
                        # split evictions across DVE and ACT so neither
                        # engine's latency throttles the PE stream
                        if g % 2 == 1:
                            nc.scalar.copy(y_sb[:, g, :], psum[:])
                        else:
                            nc.vector.tensor_copy(y_sb[:, g, :], psum[:])
                    nc.sync.dma_start(yT_v[:, :, cs], y_sb[:])

            if n_iters == 1:
                body()
            else:
                with tc.For_i(0, n_iters, 1):
                    body()
    nc.compile()
    return nc


def _get_nc():
    global _NC
    if _NC is None:
        _NC = _build_nc()
    return _NC


def _route(x, wr):
    """Replicates the reference router exactly (fp32 numpy)."""
    xt = np.transpose(x, (1, 0, 2)).reshape(T, D)  # [T, D] fp32
    logits = xt.astype(np.float32) @ wr.astype(np.float32)  # [T, E]
    m = logits.max(axis=-1, keepdims=True)
    p = np.exp(logits - m, dtype=np.float32)
    p /= p.sum(axis=-1, keepdims=True)
    top1 = np.argmax(p, axis=-1)
    p_masked = p.copy()
    p_masked[np.arange(T), top1] = -np.inf
    top2 = np.argmax(p_masked, axis=-1)
    eidx = np.stack([top1, top2], axis=1)  # [T, K]
    ew = np.take_along_axis(p, eidx, axis=1).astype(np.float32)  # [T, K]

    flat_e = eidx.reshape(-1)
    order = np.argsort(flat_e, kind="stable")
    sorted_e = flat_e[order]
    hist = np.bincount(flat_e, minlength=E)
    starts = np.cumsum(hist) - hist
    pos = np.arange(T * K) - starts[sorted_e]
    keep = pos < C
    slot = np.where(keep, sorted_e * C + pos, E * C)
    token = order // K
    return xt, ew, order, keep, slot, token


def _make_in_maps(x, wr, w1, w2):
    xt, ew, order, keep, slot, token = _route(x, wr)
    slot_token = np.zeros(E * C, np.int64)
    slot_token[slot[keep]] = token[keep]
    xT_bf = np.ascontiguousarray(xt.T.astype(BF16))  # [D, T]
    in_maps = []
    for e in range(E):
        idx = slot_token[e * C : (e + 1) * C]
        in_maps.append(
            {
                "xgT": np.ascontiguousarray(xT_bf[:, idx]),
                "w1": np.ascontiguousarray(w1[e].astype(BF16)),
                "w2": np.ascontiguousarray(w2[e].astype(BF16)),
            }
        )
    return in_maps, (ew, order, keep, slot)


def kernel(x, wr, w1, w2):
    nc = _get_nc()
    in_maps, (ew, order, keep, slot) = _make_in_maps(x, wr, w1, w2)

    res = run_bass_kernel_spmd(nc, in_maps, core_ids=list(range(N_CORES)))

    # --- combine: weighted scatter back to tokens ---
    Y = np.empty((E * C, D), np.float32)
    for e in range(E):
        Y[e * C : (e + 1) * C] = res.results[e]["yT"].T

    inv = np.empty(T * K, np.int64)
    inv[order] = np.arange(T * K)
    slot_tk = slot[inv].reshape(T, K)
    keep_tk = keep[inv].reshape(T, K)

    out_flat = np.zeros((T, D), np.float32)
    for k in range(K):
        sl = np.clip(slot_tk[:, k], 0, E * C - 1)
        contrib = Y[sl] * ew[:, k : k + 1]
        contrib[~keep_tk[:, k]] = 0.0
        out_flat += contrib
    return np.ascontiguousarray(
        out_flat.reshape(S, B, D).transpose(1, 0, 2)
    ).astype(np.float32)


# ---------------------------------------------------------------------------
# Benchmark helper (used by test.py; not part of the grading contract).
# ---------------------------------------------------------------------------


def make_bench(in_maps):
    import jax
    from jax.experimental.shard_map import shard_map
    from jax.sharding import Mesh, PartitionSpec, NamedSharding
    from concourse.bass2jax import (
        _bass_exec_p,
        install_neuronx_cc_hook,
        partition_id_tensor,
    )

    nc = _NC if _NC is not None else _get_nc()
    install_neuronx_cc_hook()
    partition_name = nc.partition_id_tensor.name if nc.partition_id_tensor else None

    in_names, out_names, out_avals, zero_outs = [], [], [], []
    for alloc in nc.m.functions[0].allocations:
        if not isinstance(alloc, mybir.MemoryLocationSet):
            continue
        name = alloc.memorylocations[0].name
        if alloc.kind == "ExternalInput":
            if name != partition_name:
                in_names.append(name)
        elif alloc.kind == "ExternalOutput":
            shape = tuple(alloc.tensor_shape)
            dtype = mybir.dt.np(alloc.dtype)
            out_avals.append(jax.core.ShapedArray(shape, dtype))
            zero_outs.append(np.zeros(shape, dtype))
            out_names.append(name)
    n_params = len(in_names)
    all_in_names = list(in_names) + list(out_names)
    if partition_name is not None:
        all_in_names.append(partition_name)
    if nc.dbg_addr is not None:
        dbg_zero = np.zeros((1, 2), np.uint32)
        in_maps = [{**m, nc.dbg_addr.name: dbg_zero} for m in in_maps]

    def _body(*args):
        operands = list(args)
        if partition_name is not None:
            operands.append(partition_id_tensor())
        outs = _bass_exec_p.bind(
            *operands,
            out_avals=tuple(out_avals),
            in_names=tuple(all_in_names),
            out_names=tuple(out_names),
            lowering_input_output_aliases=(),
            sim_require_finite=True,
            sim_require_nnan=True,
            nc=nc,
        )
        return tuple(outs)

    devices = jax.devices()[:N_CORES]
    mesh = Mesh(np.asarray(devices), ("core",))
    n_outs = len(out_names)
    in_specs = (PartitionSpec("core"),) * (n_params + n_outs)
    out_specs = (PartitionSpec("core"),) * n_outs
    fn = jax.jit(
        shard_map(_body, mesh=mesh, in_specs=in_specs, out_specs=out_specs,
                  check_rep=False),
        keep_unused=True,
    )
    concat_in = [
        np.concatenate([np.asarray(in_maps[c][name]) for c in range(N_CORES)],
                       axis=0)
        for name in in_names
    ]
    concat_zeros = [
        np.zeros((N_CORES * z.shape[0], *z.shape[1:]), z.dtype)
        for z in zero_outs
    ]
    shard = NamedSharding(mesh, PartitionSpec("core"))
    args = [jax.device_put(a, shard) for a in concat_in + concat_zeros]
    return fn, args, out_names


def benchmark(in_maps, iters=20, warmup=3):
    import time
    import jax

    fn, args, out_names = make_bench(in_maps)
    for _ in range(warmup):
        out = fn(*args)
        jax.block_until_ready(out)
    times = []
    for _ in range(iters):
        t0 = time.perf_counter()
        out = fn(*args)
        jax.block_until_ready(out)
        times.append(time.perf_counter() - t0)
    return min(times), sorted(times)[len(times) // 2], out


# revision 6
# speedup vs baseline: 274.1892x; 1.4892x over previous
"""MegablockMoE kernel for 8 Trainium2 NeuronCores.

Strategy (per sharding hint): expert-parallel. The router + token
dispatch/combine permutations (pure index bookkeeping) run on host as the
shard/unshard step; each of the 8 cores owns one expert and runs the two big
GEMMs (gelu(xg @ w1[e]) @ w2[e], 34.4 GFLOP/core) in bf16 with fp32 PSUM
accumulation, weights resident in SBUF, hT intermediate never leaving chip.

Device kernel (identical NEFF on all 8 cores, SPMD over experts):
    in : xgT [D, C] bf16   -- gathered tokens for this expert, transposed
         w1  [D, DFF] bf16, w2 [DFF, D] bf16
    mid: hT  [DFF, c_tile] bf16 = gelu(w1.T @ xgT)   (exact erf gelu, SBUF)
    out: yT  [D, C] f32    = w2.T @ hT
"""

import numpy as np
import ml_dtypes

import concourse.mybir as mybir
import concourse.tile as tile
from concourse import bacc
from concourse.bass_utils import run_bass_kernel_spmd

B, S, D = 4, 2048, 1024
E, K, DFF = 8, 2, 4096
T = B * S
C = K * T // E  # 2048 expert capacity
BF16 = ml_dtypes.bfloat16
N_CORES = 8

KO1, KO2 = D // 128, DFF // 128
W1_CH = 8            # f-chunks of w1 (separate tiles -> fine-grained DMA deps)
W1_F = DFF // W1_CH  # 512
W2_CH = 8            # o-chunks of w2
W2_O = KO2 // W2_CH  # 4

_NC = None


def _build_nc(c_tile=256, psum_bufs=4, ht_bufs=2, y_bufs=2, xg_bufs=2,
              n_iters=1, debug=True):
    nc = bacc.Bacc(None, target_bir_lowering=False, debug=debug)
    xgT = nc.dram_tensor("xgT", [D, C], mybir.dt.bfloat16, kind="ExternalInput")
    w1 = nc.dram_tensor("w1", [D, DFF], mybir.dt.bfloat16, kind="ExternalInput")
    w2 = nc.dram_tensor("w2", [DFF, D], mybir.dt.bfloat16, kind="ExternalInput")
    yT = nc.dram_tensor("yT", [D, C], mybir.dt.float32, kind="ExternalOutput")

    xgT_v = xgT.rearrange("(o p) c -> p o c", p=128)
    w1_v = w1.rearrange("(o p) f -> p o f", p=128)
    w2_v = w2.rearrange("(o p) d -> p o d", p=128)
    yT_v = yT.rearrange("(o p) c -> p o c", p=128)
    n_ct = C // c_tile

    with tile.TileContext(nc) as tc:
        with (
            tc.tile_pool(name="wpool", bufs=1) as wpool,
            tc.tile_pool(name="xpool", bufs=xg_bufs) as xpool,
            tc.tile_pool(name="hpool", bufs=ht_bufs) as hpool,
            tc.tile_pool(name="ypool", bufs=y_bufs) as ypool,
            tc.tile_pool(name="ps1", bufs=psum_bufs, space="PSUM") as ps1,
            tc.tile_pool(name="ps2", bufs=psum_bufs, space="PSUM") as ps2,
        ):
            # first xg tile before weights: small and needed immediately
            xg_tiles = {}
            if n_iters == 1:
                xg_tiles[0] = xpool.tile([128, KO1, c_tile], mybir.dt.bfloat16,
                                         tag="xg", name="xg0")
                nc.sync.dma_start(xg_tiles[0][:], xgT_v[:, :, 0:c_tile])

            w1_tiles = []
            for ch in range(W1_CH):
                wt = wpool.tile([128, KO1, W1_F], mybir.dt.bfloat16,
                                tag=f"w1_{ch}", name=f"w1t{ch}")
                nc.sync.dma_start(wt[:], w1_v[:, :, ch * W1_F : (ch + 1) * W1_F])
                w1_tiles.append(wt)
            w2_tiles = []
            for ch in range(W2_CH):
                wt = wpool.tile([128, W2_O, D], mybir.dt.bfloat16,
                                tag=f"w2_{ch}", name=f"w2t{ch}")
                nc.sync.dma_start(wt[:], w2_v[:, ch * W2_O : (ch + 1) * W2_O, :])
                w2_tiles.append(wt)

            def w1_ap(o, f):
                ch, r = divmod(f, W1_F // 128)
                return w1_tiles[ch][:, o, r * 128 : (r + 1) * 128]

            def w2_ap(f, g):
                ch, r = divmod(f, W2_O)
                return w2_tiles[ch][:, r, g * 128 : (g + 1) * 128]

            def evict_y(y_sb, g, psum):
                # split evictions across DVE and ACT so neither engine's
                # latency throttles the PE stream
                if g % 2 == 1:
                    nc.scalar.copy(y_sb[:, g, :], psum[:])
                else:
                    nc.vector.tensor_copy(y_sb[:, g, :], psum[:])

            def body(_=None):
                for t in range(n_ct):
                    cs = slice(t * c_tile, (t + 1) * c_tile)
                    if t not in xg_tiles:
                        xg_tiles[t] = xpool.tile(
                            [128, KO1, c_tile], mybir.dt.bfloat16, tag="xg",
                            name=f"xg{t}",
                        )
                        nc.sync.dma_start(xg_tiles[t][:], xgT_v[:, :, cs])
                    xg_sb = xg_tiles[t]

                    hT_sb = hpool.tile([128, KO2, c_tile], mybir.dt.bfloat16,
                                       tag="hT")
                    y_sb = ypool.tile([128, KO1, c_tile], mybir.dt.float32,
                                      tag="y")
                    # pass A: mm1(f) interleaved with mm2 for g in 0..3 —
                    # mm2's f-step consumes hT[f] right after its eviction,
                    # removing the mm1->mm2 phase boundary. 4 ps1 bufs +
                    # 4 accumulating mm2 psums = all 8 PSUM banks.
                    ps2g = [ps2.tile([128, c_tile], mybir.dt.float32,
                                     tag="p2", name=f"p2a{t}_{g}")
                            for g in range(4)]
                    for f in range(KO2):
                        psum = ps1.tile([128, c_tile], mybir.dt.float32,
                                        tag="p1")
                        for o in range(KO1):
                            nc.tensor.matmul(
                                psum[:], w1_ap(o, f), xg_sb[:, o, :],
                                start=(o == 0), stop=(o == KO1 - 1),
                            )
                        nc.scalar.activation(
                            hT_sb[:, f, :], psum[:],
                            mybir.ActivationFunctionType.Gelu,
                        )
                        for g in range(4):
                            nc.tensor.matmul(
                                ps2g[g][:], w2_ap(f, g), hT_sb[:, f, :],
                                start=(f == 0), stop=(f == KO2 - 1),
                            )
                    for g in range(4):
                        evict_y(y_sb, g, ps2g[g])
                    # pass B: mm2 for g in 4..7 (hT complete by now)
                    ps2h = [ps2.tile([128, c_tile], mybir.dt.float32,
                                     tag="p2", name=f"p2b{t}_{g}")
                            for g in range(4)]
                    for f in range(KO2):
                        for g in range(4):
                            nc.tensor.matmul(
                                ps2h[g][:], w2_ap(f, g + 4), hT_sb[:, f, :],
                                start=(f == 0), stop=(f == KO2 - 1),
                            )
                    for g in range(4):
                        evict_y(y_sb, g + 4, ps2h[g])
                    nc.sync.dma_start(yT_v[:, :, cs], y_sb[:])

            if n_iters == 1:
                body()
            else:
                with tc.For_i(0, n_iters, 1):
                    body()
    nc.compile()
    return nc


def _get_nc():
    global _NC
    if _NC is None:
        _NC = _build_nc()
    return _NC


def _route(x, wr):
    """Replicates the reference router exactly (fp32 numpy)."""
    xt = np.transpose(x, (1, 0, 2)).reshape(T, D)  # [T, D] fp32
    logits = xt.astype(np.float32) @ wr.astype(np.float32)  # [T, E]
    m = logits.max(axis=-1, keepdims=True)
    p = np.exp(logits - m, dtype=np.float32)
    p /= p.sum(axis=-1, keepdims=True)
    top1 = np.argmax(p, axis=-1)
    p_masked = p.copy()
    p_masked[np.arange(T), top1] = -np.inf
    top2 = np.argmax(p_masked, axis=-1)
    eidx = np.stack([top1, top2], axis=1)  # [T, K]
    ew = np.take_along_axis(p, eidx, axis=1).astype(np.float32)  # [T, K]

    flat_e = eidx.reshape(-1)
    order = np.argsort(flat_e, kind="stable")
    sorted_e = flat_e[order]
    hist = np.bincount(flat_e, minlength=E)
    starts = np.cumsum(hist) - hist
    pos = np.arange(T * K) - starts[sorted_e]
    keep = pos < C
    slot = np.where(keep, sorted_e * C + pos, E * C)
    token = order // K
    return xt, ew, order, keep, slot, token


def _make_in_maps(x, wr, w1, w2):
    xt, ew, order, keep, slot, token = _route(x, wr)
    slot_token = np.zeros(E * C, np.int64)
    slot_token[slot[keep]] = token[keep]
    xT_bf = np.ascontiguousarray(xt.T.astype(BF16))  # [D, T]
    in_maps = []
    for e in range(E):
        idx = slot_token[e * C : (e + 1) * C]
        in_maps.append(
            {
                "xgT": np.ascontiguousarray(xT_bf[:, idx]),
                "w1": np.ascontiguousarray(w1[e].astype(BF16)),
                "w2": np.ascontiguousarray(w2[e].astype(BF16)),
            }
        )
    return in_maps, (ew, order, keep, slot)


def kernel(x, wr, w1, w2):
    nc = _get_nc()
    in_maps, (ew, order, keep, slot) = _make_in_maps(x, wr, w1, w2)

    res = run_bass_kernel_spmd(nc, in_maps, core_ids=list(range(N_CORES)))

    # --- combine: weighted scatter back to tokens ---
    Y = np.empty((E * C, D), np.float32)
    for e in range(E):
        Y[e * C : (e + 1) * C] = res.results[e]["yT"].T

    inv = np.empty(T * K, np.int64)
    inv[order] = np.arange(T * K)
    slot_tk = slot[inv].reshape(T, K)
    keep_tk = keep[inv].reshape(T, K)

    out_flat = np.zeros((T, D), np.float32)
    for k in range(K):
        sl = np.clip(slot_tk[:, k], 0, E * C - 1)
        contrib = Y[sl] * ew[:, k : k + 1]
        contrib[~keep_tk[:, k]] = 0.0
        out_flat += contrib
    return np.ascontiguousarray(
        out_flat.reshape(S, B, D).transpose(1, 0, 2)
    ).astype(np.float32)


# ---------------------------------------------------------------------------
# Benchmark helper (used by test.py; not part of the grading contract).
# ---------------------------------------------------------------------------


def make_bench(in_maps):
    import jax
    from jax.experimental.shard_map import shard_map
    from jax.sharding import Mesh, PartitionSpec, NamedSharding
    from concourse.bass2jax import (
        _bass_exec_p,
        install_neuronx_cc_hook,
        partition_id_tensor,
    )

    nc = _NC if _NC is not None else _get_nc()
    install_neuronx_cc_hook()
    partition_name = nc.partition_id_tensor.name if nc.partition_id_tensor else None

    in_names, out_names, out_avals, zero_outs = [], [], [], []
    for alloc in nc.m.functions[0].allocations:
        if not isinstance(alloc, mybir.MemoryLocationSet):
            continue
        name = alloc.memorylocations[0].name
        if alloc.kind == "ExternalInput":
            if name != partition_name:
                in_names.append(name)
        elif alloc.kind == "ExternalOutput":
            shape = tuple(alloc.tensor_shape)
            dtype = mybir.dt.np(alloc.dtype)
            out_avals.append(jax.core.ShapedArray(shape, dtype))
            zero_outs.append(np.zeros(shape, dtype))
            out_names.append(name)
    n_params = len(in_names)
    all_in_names = list(in_names) + list(out_names)
    if partition_name is not None:
        all_in_names.append(partition_name)
    if nc.dbg_addr is not None:
        dbg_zero = np.zeros((1, 2), np.uint32)
        in_maps = [{**m, nc.dbg_addr.name: dbg_zero} for m in in_maps]

    def _body(*args):
        operands = list(args)
        if partition_name is not None:
            operands.append(partition_id_tensor())
        outs = _bass_exec_p.bind(
            *operands,
            out_avals=tuple(out_avals),
            in_names=tuple(all_in_names),
            out_names=tuple(out_names),
            lowering_input_output_aliases=(),
            sim_require_finite=True,
            sim_require_nnan=True,
            nc=nc,
        )
        return tuple(outs)

    devices = jax.devices()[:N_CORES]
    mesh = Mesh(np.asarray(devices), ("core",))
    n_outs = len(out_names)
    in_specs = (PartitionSpec("core"),) * (n_params + n_outs)
    out_specs = (PartitionSpec("core"),) * n_outs
    fn = jax.jit(
        shard_map(_body, mesh=mesh, in_specs=in_specs, out_specs=out_specs,
                  check_rep=False),
        keep_unused=True,
    )
    concat_in = [
        np.concatenate([np.asarray(in_maps[c][name]) for c in range(N_CORES)],
                       axis=0)
        for name in in_names
    ]
    concat_zeros = [
        np.zeros((N_CORES * z.shape[0], *z.shape[1:]), z.dtype)
        for z in zero_outs
    ]
    shard = NamedSharding(mesh, PartitionSpec("core"))
    args = [jax.device_put(a, shard) for a in concat_in + concat_zeros]
    return fn, args, out_names


def benchmark(in_maps, iters=20, warmup=3):
    import time
    import jax

    fn, args, out_names = make_bench(in_maps)
    for _ in range(warmup):
        out = fn(*args)
        jax.block_until_ready(out)
    times = []
    for _ in range(iters):
        t0 = time.perf_counter()
        out = fn(*args)
        jax.block_until_ready(out)
        times.append(time.perf_counter() - t0)
    return min(times), sorted(times)[len(times) // 2], out
